# revision 15
# baseline (speedup 1.0000x reference)
"""Trainium2 Bass kernel for sparse (strided) multi-head attention.

Reference computation (B=2, S=2048, H=1024, NH=16, D=64):
    q = (x @ q_w) * sigmoid(phi); k = x @ k_w; v = x @ v_w   (per-head [S, D])
    scores = q k^T / sqrt(D), masked to allowed[i, j] = (j % 4 == 0) | (|i-j| <= 8)
    out = softmax(scores) @ v;  return concat_heads(out) @ o_w + o_b

Sharding: 8 cores = 2 batches x 4 head-groups (4 heads each). Each core gets
x^T for its batch, column-sliced q/k/v weights, row-sliced o_w, and returns a
partial transposed output F^T = (attn_out_heads @ o_w_slice)^T which the host
sums over head-groups, transposes, and biases.

v5 design notes (all matmuls bf16; PSUM stays f32):
  - The +/-8 band is decomposed per 128-query SUBTILE t into
      winA: a full 128-key window [128t-8, 128t+120)  (t=0: [0,120))
      winB: a 32-key corner strip [128t+104, 128t+136), four strips packed
            onto the 128 psum partitions per 512-query block (strip s at
            partitions 32s, scored in ONE matmul via a strided lhsT AP).
    Multiplicative 0/1 post-exp masks keep exactly the reference pattern
    (winB only keeps keys >= 128t+120 to avoid double-counting the winA
    overlap; the block-3 variant drops the keys past S).
  - attn@V runs with QUERIES on the psum partition axis: lhsT = exp'd
    scores [keys, 128 q] (full-width M=128) and rhs = V[keys, 65] (64
    values + a ones column). The ones column lands the softmax denominator
    as a per-partition scalar: normalize is one DVE copy + a [128,4]
    reciprocal + one Pool mul with a free-axis broadcast -- no
    partition_broadcast, no [1,S] reciprocals, no repartition DMA.
  - PE transposes (via a shipped 128x128 identity) restore the c-major
    [channels, queries] layout; the output projection runs PER 512-QUERY
    BLOCK as soon as that block is transposed, overlapping the old serial
    tail; each block ships in one DMA issued from ACT.
  - All four heads' scores are emitted densely up front (head 1 carries
    the remaining projection waves as fillers) so ACT gets a continuous
    exp stream; the V projections (rolling one-psum-per-window, no pool
    barriers) then run as pure PE work while ACT drains, with heads
    1/0/3's attn@V woven in per block behind the V windows they need.
  - Input DMAs are split and ordered by first consumer: K weights, x
    chunks 0-1, Q weights, x chunks 2-3, V weights, masks+identity.
"""

import os
import numpy as np

B, S, H = 2, 2048, 1024
NH, D = 16, 64
PHI = 1.6180339887
STRIDE, LOCAL = 4, 8
HPG = 4              # heads per group (= per core)
GC = HPG * D         # channels per core = 256
NSK = S // STRIDE    # 512 strided keys
NT = S // 128        # 16 query subtiles
XW = S + 3 * GC      # x^T | q/k/v weights
AUXW = 3 * 128 + 64 + 256  # maskA0|maskA|maskC|maskC3|identity = 704

_CACHE = {}
LAST_RESULTS = None  # BassKernelResults of the most recent run (for profiling)


def _winA_geom(t):
    """(key-window start, window width) for subtile t's main band window.

    Window 16 is a stub holding just the last 8 keys: subtile 15's corner
    piece reads rows [0:16) of "window t+1" like everyone else."""
    if t == 16:
        return S - 8, 8
    w0 = max(0, 128 * t - LOCAL)
    return w0, (120 if t == 0 else 128)


def host_aux():
    """Masks + identity, shipped as one [128, 1408] tensor.

    maskA0 (t=0, key=kp):        |q - kp| <= 8           and kp % 4 != 0
    maskA  (t>=1, key=128t-8+kp): kp-16 <= q <= kp        and kp % 4 != 0
    maskB  (strip s rows 32s, key=128t+104+kp2, kp2=row-32s):
        kp2 >= 16 (winA overlap) and 96+kp2 <= q <= 112+kp2 and kp2 % 4 != 0
    maskB3: maskB with strip-3 rows kp2 >= 24 dropped (keys past S).
    """
    kp = np.arange(128)[:, None]
    qq = np.arange(128)[None, :]
    maskA0 = ((np.abs(qq - kp) <= LOCAL) & (kp % STRIDE != 0)).astype(np.float32)
    maskA = ((qq >= kp - 2 * LOCAL) & (qq <= kp) & (kp % STRIDE != 0)).astype(np.float32)
    kp2 = np.arange(16)[:, None]
    qh = np.arange(64)[None, :]
    mC16 = ((qh >= 48 + kp2) & (kp2 % STRIDE != 0)).astype(np.float32)
    maskC = np.zeros((128, 64), np.float32)
    maskC[0:16, :] = mC16
    maskC3 = np.zeros((128, 256), np.float32)
    for g in range(4):
        maskC3[0:16, 64 * g:64 * g + 64] = (mC16 * (kp2 < 8) if g == 3 else mC16)
    ident = np.eye(128, dtype=np.float32)
    return np.concatenate([maskA0, maskA, maskC, maskC3, ident], axis=1)


def _dtypes():
    import concourse.mybir as mybir
    name = os.environ.get("KERNEL_MM_DTYPE", "bfloat16")
    dt = {"float32": mybir.dt.float32, "float32r": mybir.dt.float32r,
          "bfloat16": mybir.dt.bfloat16}[name]
    out_dt = (mybir.dt.float32 if os.environ.get("KERNEL_OUT_F32")
              else mybir.dt.bfloat16)
    return dt, out_dt


def build_nc(loop_n=1, unroll=False):
    """Build the per-core Bass program (same NEFF for all 8 cores).

    loop_n > 1 wraps the whole pipeline in a hardware loop (benchmarking:
    wall-clock deltas between loop counts cancel dispatch overhead).
    unroll=True python-unrolls instead (for TimelineSim, which cannot
    resolve For_i branches).
    """
    import contextlib
    import concourse.mybir as mybir
    import concourse.tile as tile
    from concourse import bacc
    from collections import deque

    f32 = mybir.dt.float32
    DT, OUT_DT = _dtypes()
    AF = mybir.ActivationFunctionType

    nc = bacc.Bacc("TRN2", target_bir_lowering=False, debug=False)

    d_xin = nc.dram_tensor("xin", [H, XW], DT, kind="ExternalInput")
    d_aux = nc.dram_tensor("aux", [128, AUXW], DT, kind="ExternalInput")
    d_ow = nc.dram_tensor("ow", [GC, H], DT, kind="ExternalInput")
    d_fT = nc.dram_tensor("fT", [H, S], OUT_DT, kind="ExternalOutput")

    def mm(out, lhsT, rhs, start, stop, tile_position=None):
        nc.tensor.matmul(out, lhsT, rhs, start=start, stop=stop,
                         skip_group_check=True, tile_position=tile_position)

    with tile.TileContext(nc) as tc:
        with (
            tc.tile_pool(name="persist", bufs=1) as persist,
            tc.tile_pool(name="ph1", bufs=1) as ph1,
        ):
            sb_ow = persist.tile([128, 2, 1024], DT)

            # D-major Q^T / K^T: [128ch (2 heads), c-tile, S]
            sb_QT = persist.tile([128, 2, S], DT)
            # K^T padded by 32 zeroed cols: the packed winB score mm's last
            # strip reads 8 cols past S for block 3 (masked to zero anyway)
            sb_KT = persist.tile([128, 2, S + 32], DT)
            sb_KsT = persist.tile([128, 2, NSK], DT)      # strided keys, compacted
            # S-major V: winA windows, winB strips, strided keys; col 64 = 1.0
            sb_VshA = persist.tile([128, NT + 1, HPG, 66], DT)
            sb_Vs = persist.tile([128, NSK // 128, HPG, 66], DT)
            sb_outTs = persist.tile([128, 2, S], DT)      # c-major head outputs
            # normalized attn out, q-major: [q, block, subtile, 256 ch]
            sb_stn = persist.tile([128, 4, 4, 256], DT)

            sb_xin = ph1.tile([128, 8, XW], DT)
            sb_xT = sb_xin[:, :, 0:S]
            sb_qw = sb_xin[:, :, S:S + GC]
            sb_kw = sb_xin[:, :, S + GC:S + 2 * GC]
            sb_vw = sb_xin[:, :, S + 2 * GC:S + 3 * GC]
            sb_aux = ph1.tile([128, AUXW], DT)
            sb_mA0 = sb_aux[:, 0:128]
            sb_mA = sb_aux[:, 128:256]
            sb_mC = sb_aux[:, 256:320]
            sb_mC3 = sb_aux[:, 320:576]
            sb_ident = sb_aux[:, 576:704]

            loop_cm = (tc.For_i(0, loop_n, 1) if loop_n > 1 and not unroll
                       else contextlib.nullcontext())
            with loop_cm, (
                tc.tile_pool(name="ats", bufs=4)) as p_ats, (
                tc.tile_pool(name="atA", bufs=4)) as p_atA, (
                tc.tile_pool(name="atB", bufs=4)) as p_atB, (
                tc.tile_pool(name="stg", bufs=3)) as p_stg, (
                tc.tile_pool(name="rec", bufs=3)) as p_rec, (
                tc.tile_pool(name="stC", bufs=2)) as p_stC, (
                tc.tile_pool(name="psS", bufs=2, space="PSUM")) as psS, (
                tc.tile_pool(name="psB", bufs=4, space="PSUM")) as psB:

              for _it in range(loop_n if unroll else 1):
                # ---------------- shared emit helpers ----------------
                def hslices(h):
                    ct, pb = h // 2, (h % 2) * 64
                    return (ct, pb, sb_QT[pb:pb + 64, ct, :],
                            sb_KT[pb:pb + 64, ct, 0:S], sb_KsT[pb:pb + 64, ct, :])

                def sc_strided(h, b, at_s2):
                    """Strided scores for query block b: 4 key tiles in two
                    psum pairs, one double-width exp per pair (halves the
                    per-op ACT bubble)."""
                    ct, pb, QT, KT, KsT = hslices(h)
                    ql = slice(512 * b, 512 * (b + 1))
                    for j in range(2):
                        ps = psS.tile([128, 2, 512], f32, tag="sc", name="ps_sc")
                        for i2 in range(2):
                            i = 2 * j + i2
                            mm(ps[:, i2, :], KsT[:, 128 * i:128 * (i + 1)],
                               QT[:, ql], start=True, stop=True)
                        nc.scalar.activation(at_s2[:, 2 * j:2 * j + 2, ql],
                                             ps[:], AF.Exp)

                def band_group(h, b, at_bA, at_bB):
                    """Band scores for block b: 4 winA windows packed in one
                    psum + the 4-strip winB matmul; exp + 0/1 masks.

                    winA rows kw:128 (t=0 only) are stale psum exp'd to
                    garbage but never read; winB's out-of-range keys hit the
                    zeroed K^T pad and are masked off.
                    """
                    ct, pb, QT, KT, _ = hslices(h)
                    psA = psB.tile([128, 512], f32, tag="b", name="ps_bdA")
                    for g in range(4):
                        t = 4 * b + g
                        w0, kw_ = _winA_geom(t)
                        mm(psA[0:kw_, 128 * g:128 * (g + 1)],
                           KT[:, w0:w0 + kw_], QT[:, 128 * t:128 * (t + 1)],
                           start=True, stop=True)
                    nc.scalar.activation(at_bA[:, 512 * b:512 * (b + 1)],
                                         psA[:], AF.Exp)
                    # corner pieces: keys [128t+120, 128t+136) x queries
                    # [128t+64, 128t+128), i.e. rows [0:16) of window t+1
                    psC_ = psB.tile([128, 256], f32, tag="b", name="ps_bdC")
                    for g in range(4):
                        t = 4 * b + g
                        cw0 = 128 * t + 120
                        mm(psC_[0:16, 64 * g:64 * g + 64],
                           sb_KT[pb:pb + 64, ct, cw0:cw0 + 16],
                           QT[:, 128 * t + 64:128 * t + 128],
                           start=True, stop=True)
                    nc.scalar.activation(at_bB[:, 256 * b:256 * (b + 1)],
                                         psC_[:], AF.Exp)
                    # 0/1 masks: DVE for heads 2/3 (they chain into attn@V
                    # with little slack; gpsimd is ~3x slower per column),
                    # Pool for heads 0/1 (their attn@V runs much later)
                    eng = nc.vector if h in (2, 3) else nc.gpsimd
                    if b == 0:
                        eng.tensor_mul(at_bA[0:120, 0:128],
                                       at_bA[0:120, 0:128],
                                       sb_mA0[0:120, :])
                        sl = at_bA[:, 128:512].rearrange("p (a q) -> p a q", q=128)
                        eng.tensor_mul(sl, sl, _free_bcast(sb_mA, 3))
                    else:
                        sl = at_bA[:, 512 * b:512 * (b + 1)].rearrange(
                            "p (a q) -> p a q", q=128)
                        eng.tensor_mul(sl, sl, _free_bcast(sb_mA, 4))
                    if b == 3:
                        eng.tensor_mul(at_bB[0:16, 768:1024],
                                       at_bB[0:16, 768:1024],
                                       sb_mC3[0:16, :])
                    else:
                        sl = at_bB[0:16, 256 * b:256 * (b + 1)].rearrange(
                            "p (a q) -> p a q", q=64)
                        eng.tensor_mul(sl, sl, _free_bcast(sb_mC[0:16, :], 4))

                def emit_scores(h, at_s, at_bA, at_bB, filler):
                    """All scores for head h, interleaved per query block so
                    attn@V of block b only waits on its own exp units."""
                    for b in range(4):
                        sc_strided(h, b, at_s)
                        filler()
                        band_group(h, b, at_bA, at_bB)
                        filler()

                def av_block(h, b, at_s, at_bA, at_bB):
                    """attn@[V|1] for head h, query block b, q on partitions.

                    Per 128-q subtile g the psum columns [65g, 65g+65) hold
                    values 0:64 and the softmax denominator at 64. One DVE
                    copy drains the block, a [128,4] reciprocal and one Pool
                    mul (free-axis broadcast) write the normalized output
                    into sb_stn's channel slice for this head.
                    """
                    av = psB.tile([128, 512], f32, tag="b", name="ps_av")
                    for g in range(4):
                        qs0 = 512 * b + 128 * g
                        t = 4 * b + g
                        col = slice(65 * g, 65 * g + 65)
                        for i in range(4):
                            mm(av[:, col], at_s[:, i, qs0:qs0 + 128],
                               sb_Vs[:, i, h, 0:65],
                               start=(i == 0), stop=False)
                        w0, kw_ = _winA_geom(t)
                        mm(av[:, col], at_bA[0:kw_, qs0:qs0 + 128],
                           sb_VshA[0:kw_, t, h, 0:65], start=False, stop=False)
                        # corner: rows [0:16) of window t+1, queries [64,128)
                        mm(av[64:128, col],
                           at_bB[0:16, 256 * b + 64 * g:256 * b + 64 * g + 64],
                           sb_VshA[0:16, t + 1, h, 0:65],
                           start=False, stop=True)
                    st = p_stg.tile([128, 4, 65], DT, tag="stg", name="st_av")
                    nc.vector.tensor_copy(
                        st[:], av[:, 0:260].rearrange("p (g c) -> p g c", c=65))
                    rec = p_rec.tile([128, 4], DT, tag="rec", name="rec")
                    with nc.allow_low_precision("bf16 softmax denominators"):
                        nc.vector.reciprocal(rec[:], st[:, :, 64])
                    nc.gpsimd.tensor_mul(sb_stn[:, b, :, 64 * h:64 * h + 64],
                                         st[:, :, 0:64],
                                         _free_bcast_last(rec[:], 64))

                def tp_block(b):
                    """Transpose block b's normalized [q, ch] to c-major."""
                    for c in range(2):
                        # transpose is pass-through: psum holds bf16 here
                        tp = psB.tile([128, 512], DT, tag="b", name="ps_tp")
                        for g in range(4):
                            nc.tensor.transpose(
                                tp[:, 128 * g:128 * (g + 1)],
                                sb_stn[:, b, g, 128 * c:128 * (c + 1)],
                                sb_ident)
                        nc.vector.tensor_copy(
                            sb_outTs[:, c, 512 * b:512 * (b + 1)], tp[:])

                def out_block(b, fT_r):
                    """Output projection + DMA for query block b."""
                    stC = p_stC.tile([128, 8, 512], OUT_DT, tag="stC", name="stC")
                    for ft in range(8):
                        pc = psB.tile([128, 512], f32, tag="b", name="ps_ft")
                        for ctt in range(2):
                            mm(pc[:], sb_ow[:, ctt, 128 * ft:128 * (ft + 1)],
                               sb_outTs[:, ctt, 512 * b:512 * (b + 1)],
                               start=(ctt == 0), stop=(ctt == 1))
                        if ft % 2 == 1:
                            nc.scalar.copy(stC[:, ft, :], pc[:])
                        else:
                            nc.vector.tensor_copy(stC[:, ft, :], pc[:])
                    # issued on ACT so the SP sequencer reaches the next loop
                    # iteration's input DMAs mid-iteration
                    nc.scalar.dma_start(out=fT_r[:, :, 512 * b:512 * (b + 1)],
                                        in_=stC[:])

                # ---------------- Phase A ----------------
                if True:
                    xin_r = d_xin.rearrange("(t p) s -> p t s", p=128)
                    fT_r = d_fT.rearrange("(t p) s -> p t s", p=128)

                    # pieces ordered by first consumer: K weights, x chunks
                    # 0-1, Q weights, x chunks 2-3, V weights, masks+identity
                    def dma_cols(c0, c1):
                        nc.sync.dma_start(out=sb_xin[:, :, c0:c1],
                                          in_=xin_r[:, :, c0:c1])
                    dma_cols(S + GC, S + 2 * GC)      # kw
                    dma_cols(0, 512)
                    dma_cols(512, 1024)
                    dma_cols(S, S + GC)              # qw
                    dma_cols(1024, 1536)
                    dma_cols(1536, 2048)
                    dma_cols(S + 2 * GC, XW)         # vw
                    nc.sync.dma_start(out=sb_aux[:], in_=d_aux[:])
                    nc.sync.dma_start(out=sb_ow[:], in_=d_ow.rearrange("(t p) f -> p t f", p=128))

                    # window 16 zeroed first: v_winA(16) then writes rows
                    # [0:8); rows [8:16) must stay 0 (keys past S)
                    nc.gpsimd.memset(sb_VshA[:, NT, :, :], 0.0)
                    # ones columns for the attn@V row-sum trick; zero K^T pad
                    nc.gpsimd.memset(sb_VshA[:, :, :, 64], 1.0)
                    nc.gpsimd.memset(sb_Vs[:, :, :, 64], 1.0)
                    nc.gpsimd.memset(sb_KT[:, :, S:S + 32], 0.0)

                    def qk0_part(w_sb, w_out, ss):
                        ps = psB.tile([128, 512], f32, tag="b", name="psproj")
                        for ht in range(8):
                            mm(ps[:], w_sb[:, ht, 0:128],
                               sb_xT[:, ht, 512 * ss:512 * (ss + 1)],
                               start=(ht == 0), stop=(ht == 7))
                        nc.vector.tensor_copy(
                            w_out[:, 0, 512 * ss:512 * (ss + 1)], ps[:])

                    def ks_compact(ct):
                        ks = sb_KT[:, ct, 0:S].rearrange("p (r f) -> p r f", f=STRIDE)[:, :, 0]
                        nc.vector.tensor_copy(sb_KsT[:, ct, :], ks)

                    def qk1_wave(w_sb, w_out, wave):
                        ps = [psS.tile([128, 512], f32, tag="sc", name="ps_qk1")
                              for _ in range(2)]
                        for ht in range(8):
                            for u in range(2):
                                mm(ps[u][:], w_sb[:, ht, 128:256],
                                   sb_xT[:, ht, 1024 * wave + 512 * u:
                                         1024 * wave + 512 * (u + 1)],
                                   start=(ht == 0), stop=(ht == 7))
                        for u in range(2):
                            nc.vector.tensor_copy(
                                w_out[:, 1, 1024 * wave + 512 * u:
                                      1024 * wave + 512 * (u + 1)], ps[u][:])

                    # V projections: rolling one-psum-per-window emission
                    def v_winA(t):
                        w0, kw_ = _winA_geom(t)
                        ps = psB.tile([128, GC], f32, tag="b", name="psprojv")
                        for ht in range(8):
                            mm(ps[0:kw_, :], sb_xT[:, ht, w0:w0 + kw_],
                               sb_vw[:, ht, :], start=(ht == 0), stop=(ht == 7))
                        nc.vector.tensor_copy(
                            sb_VshA[0:kw_, t, :, 0:64],
                            ps[0:kw_, :].rearrange("p (h d) -> p h d", h=HPG))

                    def v_str(i):
                        ps = psB.tile([128, GC], f32, tag="b", name="psprojs")
                        for ht in range(8):
                            x4 = sb_xT[:, ht, :].rearrange("p (a b) -> p a b", b=STRIDE)[:, :, 0]
                            mm(ps[:], x4[:, 128 * i:128 * (i + 1)],
                               sb_vw[:, ht, :], start=(ht == 0), stop=(ht == 7))
                        nc.vector.tensor_copy(
                            sb_Vs[:, i, :, 0:64],
                            ps[:].rearrange("p (h d) -> p h d", h=HPG))

                    # K ct0 as chunks land, then head 1 scores can start
                    for ss in range(4):
                        qk0_part(sb_kw, sb_KT, ss)
                    ks_compact(0)
                    qk0_part(sb_qw, sb_QT, 0)

                    def alloc_at():
                        ats = p_ats.tile([128, 4, S], DT, tag="ats", name="at_s")
                        atA = p_atA.tile([128, S], DT, tag="atA", name="at_bA")
                        atB = p_atB.tile([128, S // 2], DT, tag="atB", name="at_bC")
                        return ats, atA, atB

                    fillers = deque()
                    fillers.append(lambda: qk0_part(sb_qw, sb_QT, 1))
                    fillers.append(lambda: qk0_part(sb_qw, sb_QT, 2))
                    fillers.append(lambda: qk0_part(sb_qw, sb_QT, 3))
                    fillers.append(lambda: qk1_wave(sb_qw, sb_QT, 0))
                    fillers.append(lambda: qk1_wave(sb_qw, sb_QT, 1))
                    fillers.append(lambda: qk1_wave(sb_kw, sb_KT, 0))
                    fillers.append(lambda: (qk1_wave(sb_kw, sb_KT, 1),
                                            ks_compact(1)))

                    def fA():
                        if fillers:
                            fillers.popleft()()

                    h1 = alloc_at()
                    emit_scores(1, *h1, fA)
                    # V projections split across heads 0 and 3's score waves
                    # so each head's score span roughly matches its exp time
                    # (ACT never starves) while the last x^T readers still
                    # finish mid-iteration (the next loop iteration's x-chunk
                    # DMAs are WAR-gated on them)
                    fillers.append(lambda: (v_str(0), v_str(1)))
                    fillers.append(lambda: (v_str(2), v_str(3)))
                    fillers.append(lambda: (v_winA(NT), v_winA(0), v_winA(1)))
                    fillers.append(lambda: (v_winA(2), v_winA(3), v_winA(4)))
                    fillers.append(lambda: (v_winA(5), v_winA(6)))
                    fillers.append(lambda: (v_winA(7), v_winA(8)))
                    h0 = alloc_at()
                    emit_scores(0, *h0, fA)
                    while fillers:
                        fillers.popleft()()

                # ---------------- Phase B ----------------
                if True:
                    h3 = alloc_at()
                    fillers.append(lambda: (v_winA(9), v_winA(10)))
                    fillers.append(lambda: (v_winA(11), v_winA(12)))
                    fillers.append(lambda: (v_winA(13), v_winA(14)))
                    fillers.append(lambda: v_winA(15))
                    emit_scores(3, *h3, fA)
                    while fillers:
                        fillers.popleft()()

                    # heads 1/0's attn@V blocks fill head 2's score waves
                    avq = deque()
                    for b in range(4):
                        avq.append(lambda b=b: av_block(1, b, *h1))
                        avq.append(lambda b=b: av_block(0, b, *h0))

                    def fB():
                        if avq:
                            avq.popleft()()

                    h2 = alloc_at()
                    emit_scores(2, *h2, fB)
                    while avq:
                        avq.popleft()()

                    # tail: heads 3/2's attn@V per block, transposes, and the
                    # block's output projection, staggered so out-proj of
                    # block b runs while block b+1's normalize chain drains
                    av_block(3, 0, *h3)
                    av_block(2, 0, *h2)
                    av_block(3, 1, *h3)
                    av_block(2, 1, *h2)
                    tp_block(0)
                    av_block(3, 2, *h3)
                    av_block(2, 2, *h2)
                    out_block(0, fT_r)
                    tp_block(1)
                    av_block(3, 3, *h3)
                    av_block(2, 3, *h2)
                    out_block(1, fT_r)
                    tp_block(2)
                    out_block(2, fT_r)
                    tp_block(3)
                    out_block(3, fT_r)

    nc.compile()
    return nc


def _free_bcast(ap, n):
    """Broadcast a [P, W] AP along a new middle free axis of length n."""
    import concourse.bass as bass
    return bass.AP(tensor=ap.tensor, offset=ap.offset,
                   ap=[list(ap.ap[0]), [0, n], list(ap.ap[1])])


def _free_bcast_last(ap, n):
    """Broadcast a [P, W] AP along a new last free axis of length n."""
    import concourse.bass as bass
    return bass.AP(tensor=ap.tensor, offset=ap.offset,
                   ap=[list(ap.ap[0]), list(ap.ap[1]), [0, n]])


def _strip_ap(ap2d, col0, stride, w, n):
    """[P, n, w] AP over a [P, W] AP: n strips of width w every `stride`
    columns starting at col0."""
    import concourse.bass as bass
    fs = ap2d.ap[1][0]
    return bass.AP(tensor=ap2d.tensor, offset=ap2d.offset + col0 * fs,
                   ap=[list(ap2d.ap[0]), [stride * fs, n], [fs, w]])


def get_nc():
    key = (os.environ.get("KERNEL_MM_DTYPE", "bfloat16"),
           bool(os.environ.get("KERNEL_OUT_F32")))
    if key not in _CACHE:
        _CACHE[key] = build_nc()
    return _CACHE[key]


def host_inputs(x, q_w, k_w, v_w, o_w, o_b, unity_scale):
    """Per-core input maps."""
    import ml_dtypes
    name = os.environ.get("KERNEL_MM_DTYPE", "bfloat16")
    np_dt = {"float32": np.float32, "float32r": np.float32,
             "bfloat16": ml_dtypes.bfloat16}[name]
    sig = 1.0 / (1.0 + np.exp(-float(np.asarray(unity_scale))))
    qw_eff = (np.asarray(q_w) * (sig / np.sqrt(D))).astype(np_dt)
    xT = np.ascontiguousarray(np.asarray(x).transpose(0, 2, 1)).astype(np_dt)
    aux = host_aux().astype(np_dt)
    k_w = np.asarray(k_w).astype(np_dt)
    v_w = np.asarray(v_w).astype(np_dt)
    o_w = np.asarray(o_w).astype(np_dt)
    in_maps = []
    for c in range(8):
        b, g = c // 4, c % 4
        cs = slice(GC * g, GC * (g + 1))
        xin = np.concatenate(
            [xT[b], qw_eff[:, cs], k_w[:, cs], v_w[:, cs]], axis=1)
        in_maps.append({
            "xin": np.ascontiguousarray(xin),
            "aux": aux,
            "ow": np.ascontiguousarray(o_w[cs, :]),
        })
    return in_maps


def kernel(x, q_w, k_w, v_w, o_w, o_b, unity_scale):
    global LAST_RESULTS
    from concourse.bass_utils import run_bass_kernel_spmd

    nc = get_nc()
    in_maps = host_inputs(x, q_w, k_w, v_w, o_w, o_b, unity_scale)
    res = run_bass_kernel_spmd(nc, in_maps, core_ids=list(range(8)),
                               trace=bool(os.environ.get("KERNEL_TRACE")))
    LAST_RESULTS = res
    out = np.zeros((B, S, H), np.float32)
    for b in range(B):
        acc = np.zeros((H, S), np.float32)
        for g in range(4):
            acc += np.asarray(res.results[4 * b + g]["fT"], np.float32)
        out[b] = acc.T
    out += np.asarray(o_b, np.float32)[None, None, :]
    return out
